# revision 7
# baseline (speedup 1.0000x reference)
"""Trainium2 Bass kernel for nn_MoEsparseRouting_81578608820387.

Reference computation (B=32, S=512, m=768, E=4 experts, TT-rank 8):
    pooled  = X.mean(axis=1)                      [B, m]
    logits  = pooled @ router_w.T + router_b      [B, E]
    gates   = gumbel_softmax_hard(logits, key=42) -> numerically exact one-hot
    base    = X @ base_w.T + base_b               [B, S, m]
    Z       = TT-chain per sample with gate-masked cores
    out     = Z * 16 + base

Key algebraic facts used here:
  * The straight-through gumbel output (y_hard - sg(y_soft) + y_soft) is
    numerically an exact one-hot in fp32 (non-argmax entries are exactly 0,
    the argmax entry is 1 within 1 ulp), so gates = one_hot(argmax(logits+g)).
    The gumbel noise g depends only on the fixed key 42 and shape [32, 4] -
    it is a compile-time constant (embedded below as exact fp32 bits).
  * For a fixed expert e, the 6-core TT chain is a linear map factoring
    through rank 8:  Z[b] = X[b] @ P[e].T @ Q[e].T  with  P[e]: [8, 768],
    Q[e]: [768, 8]  composed from the tiny cores (host-side, float64).

Sharding: data-parallel over batch, 4 samples per core across 8 cores.
Each core computes, entirely on device: the pooled means, router logits,
one-hot gates, the rank-8 TT path for all 4 experts (masked by the gates),
and the base matmul, with the TT contribution accumulated into the same
PSUM group as the base matmul.

Layout: X is fed transposed per core (XT: [768, 2048]) so the contraction
dim lands on SBUF partitions; the output is produced transposed
(OUT_T: [768, 2048]) and transposed back on the host during unsharding.
"""

import numpy as np
from contextlib import ExitStack

import concourse.bass as bass
import concourse.mybir as mybir
import concourse.tile as tile
from concourse.vector_clock import VectorClock, ScopedClock

F32 = mybir.dt.float32
F32R = mybir.dt.float32r

B, S, M = 32, 512, 768
E, R = 4, 8
ALPHA = 16.0
N_CORES = 8
BPC = B // N_CORES          # samples per core = 4
SPC = BPC * S               # s-positions per core = 2048
KT = M // 128               # 6 k tiles
JT = M // 128               # 6 output tiles

# Exact fp32 bits of -log(-log(uniform(key(42), [32,4], 1e-6, 1-1e-6))),
# matching jax.random with key 42 as used inside the reference.
_GUMBEL_BITS = [
    [1059519172, 1044667479, 1061447541, 3217675067],
    [3195790454, 1069435627, 1072337736, 1079048336],
    [1064342308, 3209271120, 1052098246, 1066704504],
    [3204585574, 3206543876, 3214385453, 3182688774],
    [1076248582, 1060531205, 1051773760, 1066802440],
    [3204612111, 3206576114, 3214446143, 3184995661],
    [1076827060, 1059613911, 1048823749, 1063901750],
    [3212451044, 1032977708, 1057610062, 3172541046],
    [1077967690, 1061763494, 3218003253, 3196248198],
    [1069017962, 1071576482, 1075220678, 1058851384],
    [1042117463, 1060465011, 1051649851, 1067166616],
    [3201157275, 3202100329, 3205603217, 3212271292],
    [1022969442, 1055544781, 3193711363, 1070536007],
    [1074770034, 1056010759, 3179831881, 1075542621],
    [1058148546, 1036553874, 1056867284, 3186553777],
    [1074391106, 1053431679, 3198087479, 1068597983],
    [1069344733, 1070932054, 1074279863, 1053268521],
    [3198376193, 1068350503, 1070031837, 1072405010],
    [1076981415, 1062239014, 3213794297, 3162584136],
    [1083525733, 1067382918, 3200358393, 3204072085],
    [3208189474, 3215583213, 3190137393, 1072497900],
    [1077229047, 1062622698, 3214769858, 3184075294],
    [1076260064, 1061044433, 1050399153, 1065180426],
    [3207044531, 3213201533, 3175819877, 1079536598],
    [1063709918, 3212691290, 1035989539, 1056171771],
    [3180768625, 1075642831, 1058170047, 1036032694],
    [1056410846, 3189082723, 1074534808, 1057365476],
    [1008038905, 1081166650, 1065156466, 3207147926],
    [3213423152, 1016632126, 1082345538, 1065657498],
    [3207599907, 3217629888, 3198553863, 1068112256],
    [1069572864, 1073285454, 1089469029, 1066783927],
    [3203528127, 3207612731, 3217756156, 3195337360],
]
GUMBEL = np.array(_GUMBEL_BITS, dtype=np.uint32).view(np.float32)


class _SplitDrainTC(tile.TileContext):
    """The installed walrus build rejects >2 sync-waits on one CTRL
    instruction; split the kernel-tail drain into one drain per proc."""

    def _drain_and_barrier(self, tick_clock, wait_clock):
        gc = tick_clock.global_clock
        nprocs = len(gc)
        active = [(p, gc[p]) for p in range(nprocs) if gc[p] > 0]
        for p, t in active:
            vc = VectorClock([0] * nprocs)
            vc.require_at_least(p, t)
            d = self.nc.sync.drain()
            wait_clock.add_sem_waits(d.ins, ScopedClock({None: vc}))
        self.nc.all_engine_barrier()
        assert self.sems is not None
        popped = self.nc._tile_sem_poison_stack.pop()
        assert popped is self._sem_poison
        self.nc.clear_and_free_semaphores(list(self.sems.allocated().values()))
        self.nc.all_engine_barrier()


def _split_sync_waits(nc, max_waits=1):
    """Walrus in this container rejects instructions carrying more than
    ~2 semaphore waits (1 for matmuls, whose waits ride on the LDWEIGHTS
    S3_LW encoding); offload overflow waits onto inserted NOPs that
    execute immediately before on the same engine."""
    cnt = 0
    for f in nc.m.functions:
        for bb in f.blocks:
            insts = bb.instructions
            i = 0
            while i < len(insts):
                inst = insts[i]
                max_w = 1 if isinstance(inst, mybir.InstMatmult) else max_waits
                si = inst.sync_info
                if si is not None and si.on_wait and len(si.on_wait) > max_w:
                    waits = list(si.on_wait)
                    keep = waits[-max_w:]
                    overflow = waits[:-max_w]
                    si.on_wait = keep
                    pos = i
                    for j in range(0, len(overflow), max_waits):
                        chunk = overflow[j:j + max_waits]
                        cnt += 1
                        nop = mybir.InstNoOp(
                            name=f"I-waitsplit-{cnt}",
                            engine=inst.engine,
                            ins=[], outs=[],
                            sync_info=mybir.SyncInfo(on_wait=chunk,
                                                     on_update=[]))
                        insts.insert(pos, nop)
                        pos += 1
                        i += 1
                i += 1
    return cnt


def build_nc(reps: int = 1, mm_dtype=F32R):
    """Build the per-core Bass module.

    reps > 1 wraps the body in a hardware loop (for benchmarking only).
    """
    nc = bass.Bass("TRN2", target_bir_lowering=False, debug=False,
                   num_devices=N_CORES)
    xt_d = nc.declare_dram_parameter("xt", [M, SPC], mm_dtype, isOutput=False)
    wt_d = nc.declare_dram_parameter("wt", [M, M + 4 * R], mm_dtype, isOutput=False)
    qs_d = nc.declare_dram_parameter("qs", [4 * R, M], mm_dtype, isOutput=False)
    rw_d = nc.declare_dram_parameter("rw", [M, E], F32, isOutput=False)
    bb_d = nc.declare_dram_parameter("bb", [JT, 128, 1], F32, isOutput=False)
    sm_d = nc.declare_dram_parameter("sm", [E, 8 + 4 * R], F32, isOutput=False)
    out_d = nc.declare_dram_parameter("outT", [M, SPC], F32, isOutput=True)

    def body(tc, ctx):
        cpool = ctx.enter_context(tc.tile_pool(name="consts", bufs=1))
        xpool = ctx.enter_context(tc.tile_pool(name="x", bufs=1))
        spool = ctx.enter_context(tc.tile_pool(name="small", bufs=1))
        vpool = ctx.enter_context(tc.tile_pool(name="vg", bufs=1))
        opool = ctx.enter_context(tc.tile_pool(name="outs", bufs=2))
        ps_small = ctx.enter_context(tc.tile_pool(name="ps_s", bufs=1, space="PSUM"))
        ps_v = ctx.enter_context(tc.tile_pool(name="ps_v", bufs=2, space="PSUM"))
        ps_o = ctx.enter_context(tc.tile_pool(name="ps_o", bufs=3, space="PSUM"))

        # ---- loads ----
        xt = []
        wt = []
        rw = []
        for kt in range(KT):
            t = xpool.tile([128, SPC], mm_dtype, tag=f"xt{kt}")
            nc.sync.dma_start(t[:], xt_d[kt * 128:(kt + 1) * 128, :])
            xt.append(t)
            w = cpool.tile([128, M + 4 * R], mm_dtype, tag=f"wt{kt}")
            nc.sync.dma_start(w[:], wt_d[kt * 128:(kt + 1) * 128, :])
            wt.append(w)
            r = cpool.tile([128, E], F32, tag=f"rw{kt}")
            nc.sync.dma_start(r[:], rw_d[kt * 128:(kt + 1) * 128, :])
            rw.append(r)
        qs = cpool.tile([4 * R, M], mm_dtype, tag="qs")
        nc.sync.dma_start(qs[:], qs_d[:])
        bb = []
        for j in range(JT):
            t = cpool.tile([128, 1], F32, tag=f"bb{j}")
            nc.sync.dma_start(t[:], bb_d[j])
            bb.append(t)
        sm = cpool.tile([E, 8 + 4 * R], F32, tag="sm")
        nc.sync.dma_start(sm[:], sm_d[:])
        gbias = sm[:, 0:4]
        eye4 = sm[:, 4:8]
        e8t = sm[:, 8:8 + 4 * R]

        # ---- gating ----
        pooled = []
        for kt in range(KT):
            p = spool.tile([128, BPC], F32, tag=f"pool{kt}")
            for s in range(BPC):
                nc.vector.reduce_sum(p[:, s:s + 1],
                                     xt[kt][:, s * S:(s + 1) * S].bitcast(F32),
                                     axis=mybir.AxisListType.X)
            pooled.append(p)
        z_ps = ps_small.tile([E, E], F32, tag="zps")
        for kt in range(KT):
            nc.tensor.matmul(z_ps[:], pooled[kt][:], rw[kt][:],
                             start=(kt == 0), stop=(kt == KT - 1))
        z = spool.tile([E, E], F32, tag="z")
        nc.vector.tensor_add(z[:], z_ps[:], gbias)
        rmax = spool.tile([E, 1], F32, tag="rmax")
        nc.vector.reduce_max(rmax[:], z[:], axis=mybir.AxisListType.X)
        yh = spool.tile([E, E], F32, tag="yh")
        nc.vector.tensor_scalar(yh[:], z[:], rmax[:, 0:1], None,
                                op0=mybir.AluOpType.is_equal)
        yt_ps = ps_small.tile([E, E], F32, tag="ytps")
        nc.tensor.transpose(yt_ps[:], yh[:], eye4)
        yt = spool.tile([E, E], F32, tag="yt")
        nc.scalar.copy(yt[:], yt_ps[:])
        mask_ps = ps_small.tile([4 * R, BPC], F32, tag="mps")
        nc.tensor.matmul(mask_ps[:], e8t, yt[:], start=True, stop=True)
        mask = spool.tile([4 * R, BPC], F32, tag="mask")
        nc.scalar.copy(mask[:], mask_ps[:])

        # ---- V = P_all @ x  (all experts), then gate-mask ----
        vg = []
        for s in range(BPC):
            v_ps = ps_v.tile([4 * R, S], F32, tag="vps")
            for kt in range(KT):
                nc.tensor.matmul(v_ps[:],
                                 wt[kt][:, M:M + 4 * R],
                                 xt[kt][:, s * S:(s + 1) * S],
                                 start=(kt == 0), stop=(kt == KT - 1))
            g = vpool.tile([4 * R, S], mm_dtype, tag=f"vg{s}")
            nc.vector.tensor_scalar(g[:], v_ps[:], mask[:, s:s + 1], None,
                                    op0=mybir.AluOpType.mult)
            vg.append(g)

        # ---- base matmul + TT rank-8 accumulation, bias, store ----
        for j in range(JT):
            jsl = slice(j * 128, (j + 1) * 128)
            o_sb = opool.tile([128, SPC], F32, tag="osb")
            for s in range(BPC):
                o_ps = ps_o.tile([128, S], F32, tag="ops")
                for kt in range(KT):
                    nc.tensor.matmul(o_ps[:],
                                     wt[kt][:, jsl],
                                     xt[kt][:, s * S:(s + 1) * S],
                                     start=(kt == 0), stop=False)
                nc.tensor.matmul(o_ps[:], qs[:, jsl], vg[s][:],
                                 start=False, stop=True)
                nc.scalar.activation(o_sb[:, s * S:(s + 1) * S], o_ps[:],
                                     mybir.ActivationFunctionType.Identity,
                                     bias=bb[j][:, 0:1], scale=1.0)
            nc.sync.dma_start(out_d[jsl, :], o_sb[:])

    with _SplitDrainTC(nc) as tc:
        with ExitStack() as ctx:
            if reps == 1:
                body(tc, ctx)
            else:
                with tc.For_i(0, reps, 1):
                    with ExitStack() as c2:
                        body(tc, c2)
    _split_sync_waits(nc)
    return nc


def _compose_pq(inputs):
    """Compose per-expert rank-8 factors from the TT cores (float64)."""
    c = [inputs[f"core{i}"].astype(np.float64) for i in range(6)]
    P = np.zeros((E, R, M))
    Q = np.zeros((E, M, R))
    for e in range(E):
        P[e] = np.einsum('cp,pbq,qat->tabc', c[0][e, 0], c[1][e],
                         c[2][e]).reshape(R, M)
        Q[e] = np.einsum('tap,pbq,qc->abct', c[3][e], c[4][e],
                         c[5][e, :, :, 0]).reshape(M, R)
    return P, Q


def make_in_maps(inputs):
    X = np.ascontiguousarray(inputs["X"], dtype=np.float32)
    base_w = np.asarray(inputs["base_w"], dtype=np.float32)
    base_b = np.asarray(inputs["base_b"], dtype=np.float32)
    router_w = np.asarray(inputs["router_w"], dtype=np.float32)
    router_b = np.asarray(inputs["router_b"], dtype=np.float32)

    P, Q = _compose_pq(inputs)
    p_allT = np.ascontiguousarray(P.reshape(E * R, M).T.astype(np.float32))
    qs = np.ascontiguousarray(
        (Q * ALPHA).transpose(0, 2, 1).reshape(E * R, M).astype(np.float32))
    rwT = np.ascontiguousarray((router_w / float(S)).T.astype(np.float32))
    wt = np.ascontiguousarray(
        np.concatenate([base_w.T, p_allT], axis=1).astype(np.float32))
    bb = np.ascontiguousarray(base_b.reshape(JT, 128, 1))
    e8t = np.zeros((E, E * R), np.float32)
    for e in range(E):
        e8t[e, e * R:(e + 1) * R] = 1.0

    in_maps = []
    for c in range(N_CORES):
        bs = slice(c * BPC, (c + 1) * BPC)
        xt = np.ascontiguousarray(X[bs].reshape(SPC, M).T)
        gbias = router_b[None, :] + GUMBEL[bs]
        sm = np.concatenate([gbias, np.eye(E, dtype=np.float32), e8t],
                            axis=1).astype(np.float32)
        in_maps.append({"xt": xt, "wt": wt, "qs": qs, "rw": rwT,
                        "bb": bb, "sm": np.ascontiguousarray(sm)})
    return in_maps


def unshard(results):
    out = np.empty((B, S, M), np.float32)
    for c in range(N_CORES):
        out[c * BPC:(c + 1) * BPC] = \
            results[c]["outT"].T.reshape(BPC, S, M)
    return out


_NC_CACHE = {}


def kernel(**inputs):
    from concourse.bass_utils import run_bass_kernel_spmd
    if "nc" not in _NC_CACHE:
        _NC_CACHE["nc"] = build_nc()
    nc = _NC_CACHE["nc"]
    in_maps = make_in_maps(inputs)
    res = run_bass_kernel_spmd(nc, in_maps, list(range(N_CORES)))
    return unshard(res.results)


# revision 13
# speedup vs baseline: 9.2347x; 9.2347x over previous
"""Trainium2 Bass kernel for nn_MoEsparseRouting_81578608820387.

Reference computation (B=32, S=512, m=768, E=4 experts, TT-rank 8):
    pooled  = X.mean(axis=1)                      [B, m]
    logits  = pooled @ router_w.T + router_b      [B, E]
    gates   = gumbel_softmax_hard(logits, key=42) -> numerically exact one-hot
    base    = X @ base_w.T + base_b               [B, S, m]
    Z       = TT-chain per sample with gate-masked cores
    out     = Z * 16 + base

Key algebraic facts used here:
  * The straight-through gumbel output (y_hard - sg(y_soft) + y_soft) is
    numerically an exact one-hot in fp32 (non-argmax entries are exactly 0,
    the argmax entry is 1 within 1 ulp), so gates = one_hot(argmax(logits+g)).
    The gumbel noise g depends only on the fixed key 42 and shape [32, 4] -
    it is a compile-time constant (embedded below as exact fp32 bits).
  * For a fixed expert e, the 6-core TT chain is a linear map factoring
    through rank 8:  Z[b] = X[b] @ P[e].T @ Q[e].T  with  P[e]: [8, 768],
    Q[e]: [768, 8]  composed from the tiny cores (host-side, float64).

Sharding: data-parallel over batch, 4 samples per core across 8 cores.
Each core computes, entirely on device: the pooled means, router logits,
one-hot gates, the rank-8 TT path for all 4 experts (masked by the gates),
and the base matmul, with the TT contribution accumulated into the same
PSUM group as the base matmul.

Layout: X is fed transposed per core (XT: [768, 2048]) so the contraction
dim lands on SBUF partitions; the output is produced transposed
(OUT_T: [768, 2048]) and transposed back on the host during unsharding.
"""

import numpy as np
from contextlib import ExitStack

import concourse.bass as bass
import concourse.mybir as mybir
import concourse.tile as tile
from concourse.vector_clock import VectorClock, ScopedClock

F32 = mybir.dt.float32
F32R = mybir.dt.float32r

B, S, M = 32, 512, 768
E, R = 4, 8
ALPHA = 16.0
N_CORES = 8
BPC = B // N_CORES          # samples per core = 4
SPC = BPC * S               # s-positions per core = 2048
KT = M // 128               # 6 k tiles
JT = M // 128               # 6 output tiles

# Exact fp32 bits of -log(-log(uniform(key(42), [32,4], 1e-6, 1-1e-6))),
# matching jax.random with key 42 as used inside the reference.
_GUMBEL_BITS = [
    [1059519172, 1044667479, 1061447541, 3217675067],
    [3195790454, 1069435627, 1072337736, 1079048336],
    [1064342308, 3209271120, 1052098246, 1066704504],
    [3204585574, 3206543876, 3214385453, 3182688774],
    [1076248582, 1060531205, 1051773760, 1066802440],
    [3204612111, 3206576114, 3214446143, 3184995661],
    [1076827060, 1059613911, 1048823749, 1063901750],
    [3212451044, 1032977708, 1057610062, 3172541046],
    [1077967690, 1061763494, 3218003253, 3196248198],
    [1069017962, 1071576482, 1075220678, 1058851384],
    [1042117463, 1060465011, 1051649851, 1067166616],
    [3201157275, 3202100329, 3205603217, 3212271292],
    [1022969442, 1055544781, 3193711363, 1070536007],
    [1074770034, 1056010759, 3179831881, 1075542621],
    [1058148546, 1036553874, 1056867284, 3186553777],
    [1074391106, 1053431679, 3198087479, 1068597983],
    [1069344733, 1070932054, 1074279863, 1053268521],
    [3198376193, 1068350503, 1070031837, 1072405010],
    [1076981415, 1062239014, 3213794297, 3162584136],
    [1083525733, 1067382918, 3200358393, 3204072085],
    [3208189474, 3215583213, 3190137393, 1072497900],
    [1077229047, 1062622698, 3214769858, 3184075294],
    [1076260064, 1061044433, 1050399153, 1065180426],
    [3207044531, 3213201533, 3175819877, 1079536598],
    [1063709918, 3212691290, 1035989539, 1056171771],
    [3180768625, 1075642831, 1058170047, 1036032694],
    [1056410846, 3189082723, 1074534808, 1057365476],
    [1008038905, 1081166650, 1065156466, 3207147926],
    [3213423152, 1016632126, 1082345538, 1065657498],
    [3207599907, 3217629888, 3198553863, 1068112256],
    [1069572864, 1073285454, 1089469029, 1066783927],
    [3203528127, 3207612731, 3217756156, 3195337360],
]
GUMBEL = np.array(_GUMBEL_BITS, dtype=np.uint32).view(np.float32)


class _SplitDrainTC(tile.TileContext):
    """The installed walrus build rejects >2 sync-waits on one CTRL
    instruction; split the kernel-tail drain into one drain per proc."""

    def _drain_and_barrier(self, tick_clock, wait_clock):
        gc = tick_clock.global_clock
        nprocs = len(gc)
        active = [(p, gc[p]) for p in range(nprocs) if gc[p] > 0]
        for p, t in active:
            vc = VectorClock([0] * nprocs)
            vc.require_at_least(p, t)
            d = self.nc.sync.drain()
            wait_clock.add_sem_waits(d.ins, ScopedClock({None: vc}))
        self.nc.all_engine_barrier()
        assert self.sems is not None
        popped = self.nc._tile_sem_poison_stack.pop()
        assert popped is self._sem_poison
        self.nc.clear_and_free_semaphores(list(self.sems.allocated().values()))
        self.nc.all_engine_barrier()


def _split_sync_waits(nc, max_waits=1):
    """Walrus in this container rejects instructions carrying more than
    ~2 semaphore waits (1 for matmuls, whose waits ride on the LDWEIGHTS
    S3_LW encoding); offload overflow waits onto inserted NOPs that
    execute immediately before on the same engine."""
    cnt = 0
    for f in nc.m.functions:
        for bb in f.blocks:
            insts = bb.instructions
            i = 0
            while i < len(insts):
                inst = insts[i]
                max_w = 1 if isinstance(inst, mybir.InstMatmult) else max_waits
                si = inst.sync_info
                if si is not None and si.on_wait and len(si.on_wait) > max_w:
                    waits = list(si.on_wait)
                    keep = waits[-max_w:]
                    overflow = waits[:-max_w]
                    si.on_wait = keep
                    pos = i
                    for j in range(0, len(overflow), max_waits):
                        chunk = overflow[j:j + max_waits]
                        cnt += 1
                        nop = mybir.InstNoOp(
                            name=f"I-waitsplit-{cnt}",
                            engine=inst.engine,
                            ins=[], outs=[],
                            sync_info=mybir.SyncInfo(on_wait=chunk,
                                                     on_update=[]))
                        insts.insert(pos, nop)
                        pos += 1
                        i += 1
                i += 1
    return cnt


def build_nc(reps: int = 1, mm_dtype=F32R):
    """Build the per-core Bass module.

    reps > 1 wraps the body in a hardware loop (for benchmarking only).
    """
    nc = bass.Bass("TRN2", target_bir_lowering=False, debug=False,
                   num_devices=N_CORES)
    xt_d = nc.declare_dram_parameter("xt", [M, SPC], mm_dtype, isOutput=False)
    wt_d = nc.declare_dram_parameter("wt", [M, M + 4 * R], mm_dtype, isOutput=False)
    qs_d = nc.declare_dram_parameter("qs", [4 * R, M], mm_dtype, isOutput=False)
    rw_d = nc.declare_dram_parameter("rw", [M, E], F32, isOutput=False)
    bb_d = nc.declare_dram_parameter("bb", [JT, 128, 1], F32, isOutput=False)
    sm_d = nc.declare_dram_parameter("sm", [E, 16 + 4 * R], F32, isOutput=False)
    out_d = nc.declare_dram_parameter("outT", [M, SPC], F32, isOutput=True)

    def body(tc, ctx):
        cpool = ctx.enter_context(tc.tile_pool(name="consts", bufs=1))
        xpool = ctx.enter_context(tc.tile_pool(name="x", bufs=1))
        spool = ctx.enter_context(tc.tile_pool(name="small", bufs=1))
        vpool = ctx.enter_context(tc.tile_pool(name="vg", bufs=1))
        opool = ctx.enter_context(tc.tile_pool(name="outs", bufs=4))
        ps_small = ctx.enter_context(tc.tile_pool(name="ps_s", bufs=1, space="PSUM"))
        ps_v = ctx.enter_context(tc.tile_pool(name="ps_v", bufs=1, space="PSUM"))
        ps_o = ctx.enter_context(tc.tile_pool(name="ps_o", bufs=4, space="PSUM"))

        # ---- weight/constant loads (issued first) ----
        wt = []
        rw = []
        for kt in range(KT):
            w = cpool.tile([128, M + 4 * R], mm_dtype, tag=f"wt{kt}")
            nc.sync.dma_start(w[:], wt_d[kt * 128:(kt + 1) * 128, :])
            wt.append(w)
            r = cpool.tile([128, E], F32, tag=f"rw{kt}")
            nc.sync.dma_start(r[:], rw_d[kt * 128:(kt + 1) * 128, :])
            rw.append(r)
        qs = cpool.tile([4 * R, M], mm_dtype, tag="qs")
        nc.sync.dma_start(qs[:], qs_d[:])
        bb = []
        for j in range(JT):
            t = cpool.tile([128, 1], F32, tag=f"bb{j}")
            nc.sync.dma_start(t[:], bb_d[j])
            bb.append(t)
        sm = cpool.tile([E, 16 + 4 * R], F32, tag="sm")
        nc.sync.dma_start(sm[:], sm_d[:])
        e8t = sm[:, 16:16 + 4 * R]

        # ---- X chunk loads, sample-major ----
        xc = [[None] * KT for _ in range(BPC)]
        for s in range(BPC):
            for kt in range(KT):
                t = xpool.tile([128, S], mm_dtype, tag=f"x{kt}_{s}")
                nc.sync.dma_start(
                    t[:], xt_d[kt * 128:(kt + 1) * 128, s * S:(s + 1) * S])
                xc[s][kt] = t

        # ---- per-sample: gating -> V -> fused output groups ----
        # PE stream order: gating+V for sample s+1 is emitted behind the
        # base-matmul groups of sample s, so the tiny gating matmuls (which
        # wait on DVE reductions) never stall the dense matmul stream.
        def gating_and_v(s):
            pooled = []
            for kt in range(KT):
                p = spool.tile([128, 1], F32, tag=f"p{kt}_{s}")
                nc.vector.reduce_sum(p[:], xc[s][kt][:].bitcast(F32),
                                     axis=mybir.AxisListType.X)
                pooled.append(p)
            z_ps = ps_small.tile([1, E], F32, tag="zps")
            for kt in range(KT):
                nc.tensor.matmul(z_ps[:], pooled[kt][:], rw[kt][:],
                                 start=(kt == 0), stop=(kt == KT - 1))
            z = spool.tile([1, E], F32, tag=f"z{s}")
            nc.vector.tensor_add(z[:], z_ps[:], sm[0:1, s * 4:(s + 1) * 4])
            rmax = spool.tile([1, 1], F32, tag=f"rmax{s}")
            nc.vector.reduce_max(rmax[:], z[:], axis=mybir.AxisListType.X)
            yh = spool.tile([1, E], F32, tag=f"yh{s}")
            nc.vector.tensor_scalar(yh[:], z[:], rmax[:, 0:1], None,
                                    op0=mybir.AluOpType.is_equal)
            yt_ps = ps_small.tile([E, 1], F32, tag="ytps")
            nc.tensor.transpose(yt_ps[:], yh[:], sm[0:1, 16:17])
            yt = spool.tile([E, 1], F32, tag=f"yt{s}")
            nc.vector.tensor_copy(yt[:], yt_ps[:])
            mask_ps = ps_small.tile([4 * R, 1], F32, tag="mps")
            nc.tensor.matmul(mask_ps[:], e8t, yt[:], start=True, stop=True)
            mask = spool.tile([4 * R, 1], F32, tag=f"mask{s}")
            nc.vector.tensor_copy(mask[:], mask_ps[:])

            v_ps = ps_v.tile([4 * R, S], F32, tag="vps")
            for kt in range(KT):
                nc.tensor.matmul(v_ps[:], wt[kt][:, M:M + 4 * R],
                                 xc[s][kt][:],
                                 start=(kt == 0), stop=(kt == KT - 1))
            vg = vpool.tile([4 * R, S], mm_dtype, tag=f"vg{s}")
            nc.vector.tensor_scalar(vg[:], v_ps[:], mask[:, 0:1], None,
                                    op0=mybir.AluOpType.mult)
            return vg

        vg = gating_and_v(0)
        for s in range(BPC):
            for j in range(JT):
                jsl = slice(j * 128, (j + 1) * 128)
                o_ps = ps_o.tile([128, S], F32, tag="ops")
                for kt in range(KT):
                    nc.tensor.matmul(o_ps[:], wt[kt][:, jsl], xc[s][kt][:],
                                     start=(kt == 0), stop=False)
                nc.tensor.matmul(o_ps[:], qs[:, jsl], vg[:],
                                 start=False, stop=True)
                o_sb = opool.tile([128, S], F32, tag="osb")
                nc.scalar.activation(o_sb[:], o_ps[:],
                                     mybir.ActivationFunctionType.Identity,
                                     bias=bb[j][:, 0:1], scale=1.0)
                # store on the ACT HWDGE queue set - keeps output traffic
                # out of the input queues (no head-of-line blocking)
                nc.scalar.dma_start(out_d[jsl, s * S:(s + 1) * S], o_sb[:])
            if s + 1 < BPC:
                vg = gating_and_v(s + 1)

    with _SplitDrainTC(nc) as tc:
        with ExitStack() as ctx:
            if reps == 1:
                body(tc, ctx)
            else:
                with tc.For_i(0, reps, 1):
                    with ExitStack() as c2:
                        body(tc, c2)
    _split_sync_waits(nc)
    return nc


def _compose_pq(inputs):
    """Compose per-expert rank-8 factors from the TT cores (float64)."""
    c = [inputs[f"core{i}"].astype(np.float64) for i in range(6)]
    P = np.zeros((E, R, M))
    Q = np.zeros((E, M, R))
    for e in range(E):
        P[e] = np.einsum('cp,pbq,qat->tabc', c[0][e, 0], c[1][e],
                         c[2][e]).reshape(R, M)
        Q[e] = np.einsum('tap,pbq,qc->abct', c[3][e], c[4][e],
                         c[5][e, :, :, 0]).reshape(M, R)
    return P, Q


def make_in_maps(inputs):
    X = np.ascontiguousarray(inputs["X"], dtype=np.float32)
    base_w = np.asarray(inputs["base_w"], dtype=np.float32)
    base_b = np.asarray(inputs["base_b"], dtype=np.float32)
    router_w = np.asarray(inputs["router_w"], dtype=np.float32)
    router_b = np.asarray(inputs["router_b"], dtype=np.float32)

    P, Q = _compose_pq(inputs)
    p_allT = np.ascontiguousarray(P.reshape(E * R, M).T.astype(np.float32))
    qs = np.ascontiguousarray(
        (Q * ALPHA).transpose(0, 2, 1).reshape(E * R, M).astype(np.float32))
    rwT = np.ascontiguousarray((router_w / float(S)).T.astype(np.float32))
    wt = np.ascontiguousarray(
        np.concatenate([base_w.T, p_allT], axis=1).astype(np.float32))
    bb = np.ascontiguousarray(base_b.reshape(JT, 128, 1))
    e8t = np.zeros((E, E * R), np.float32)
    for e in range(E):
        e8t[e, e * R:(e + 1) * R] = 1.0

    in_maps = []
    for c in range(N_CORES):
        bs = slice(c * BPC, (c + 1) * BPC)
        xt = np.ascontiguousarray(X[bs].reshape(SPC, M).T)
        gbias = (router_b[None, :] + GUMBEL[bs]).astype(np.float32)
        sm = np.zeros((E, 16 + E * R), np.float32)
        sm[0, 0:16] = gbias.reshape(-1)
        sm[:, 16:] = e8t
        in_maps.append({"xt": xt, "wt": wt, "qs": qs, "rw": rwT,
                        "bb": bb, "sm": np.ascontiguousarray(sm)})
    return in_maps


def unshard(results):
    out = np.empty((B, S, M), np.float32)
    for c in range(N_CORES):
        out[c * BPC:(c + 1) * BPC] = \
            results[c]["outT"].T.reshape(BPC, S, M)
    return out


_NC_CACHE = {}


def kernel(**inputs):
    from concourse.bass_utils import run_bass_kernel_spmd
    if "nc" not in _NC_CACHE:
        _NC_CACHE["nc"] = build_nc()
    nc = _NC_CACHE["nc"]
    in_maps = make_in_maps(inputs)
    res = run_bass_kernel_spmd(nc, in_maps, list(range(N_CORES)))
    return unshard(res.results)
